# revision 72
# baseline (speedup 1.0000x reference)
"""AttentionCropLayer Trainium2 kernel.

Per sample b: offsets (w,h) = floor(clip(locs[b]*224, 44, 180) - 44); output
out[b] = images[b, :, w:w+88, h:h+88] * mask, with mask the fixed 88x88
sigmoid-profile outer product.

Strategy (pure data parallel, 8 cores x 16 samples):
  - host precomputes per-sample flat element offsets (exact same f32 ops as
    the reference) plus tiny constant tables for the mask
  - device, per chunk of 8 samples (=128 partitions, partition = (sample,
    channel)): per sample one dynamic-offset HWDGE read DMA (samples
    alternate the two HWDGE rings so every SDMA engine interleaves two
    descriptor streams), each descriptor an exact 352B crop row; per chunk
    one in-place DVE mask-multiply (mask replicated on-chip via an
    outer-product matmul) and one contiguous 3.96MB store
"""

import sys

if "/opt/trn_rl_repo" not in sys.path:
    sys.path.insert(0, "/opt/trn_rl_repo")

import numpy as np

import concourse.bass as bass
import concourse.bacc as bacc
import concourse.mybir as mybir
from concourse import tile
from concourse.bass_utils import run_bass_kernel_spmd

TL = 44
CROP = 2 * TL          # 88
SCALE = 224.0
B, C, IN = 128, 16, 224
NCORES = 8
BPC = B // NCORES      # 16 samples per core
BLK = 8                # samples per chunk -> BLK*C = 128 partitions
NBLK = BPC // BLK      # 2 chunks per core
FREE = CROP * CROP     # 7744
NG = 8                 # row groups for the SWDGE wrap samples (8*16 = 128)
GR = CROP // NG        # 11 rows per group
WRUN = (GR - 1) * IN + CROP  # 2328: contiguous wrap run per (c, g)
NWRAP = 2              # samples read via SWDGE wrap layout
MAXOFF = IN - CROP     # 136
IMSZ = C * IN * IN
CHSZ = IN * IN
MAXEOFF = (BPC - 1) * IMSZ + MAXOFF * IN + MAXOFF

_nc_cache = {}


def _build_nc():
    nc = bacc.Bacc(None)
    images = nc.declare_dram_parameter(
        "images", [BPC, C, IN, IN], mybir.dt.float32, isOutput=False
    )
    offs = nc.declare_dram_parameter(
        "offs", [1, BPC], mybir.dt.int32, isOutput=False
    )
    maskrow = nc.declare_dram_parameter(
        "maskrow", [1, FREE], mybir.dt.float32, isOutput=False
    )
    ones1 = nc.declare_dram_parameter(
        "ones1", [1, BLK * C], mybir.dt.float32, isOutput=False
    )
    # tables for the two SWDGE wrap-read samples: partition p = c*NG + g
    wraprows = nc.declare_dram_parameter(
        "wraprows", [NG, GR * CROP], mybir.dt.float32, isOutput=False
    )
    selwrap = nc.declare_dram_parameter(
        "selwrap", [NG, BLK * C], mybir.dt.float32, isOutput=False
    )
    out = nc.declare_dram_parameter(
        "out", [BPC, C, CROP, CROP], mybir.dt.float32, isOutput=True
    )

    with tile.TileContext(nc) as tc:
        with (
            tc.tile_pool(name="const", bufs=1) as cpool,
            tc.tile_pool(name="work", bufs=2) as wpool,
            tc.tile_pool(name="wrapp", bufs=1) as wrappool,
            tc.tile_pool(name="psum", bufs=2, space="PSUM") as ppool,
        ):
            offs_sb = cpool.tile([1, BPC], mybir.dt.int32)
            nc.sync.dma_start(out=offs_sb[:], in_=offs[:])
            wrow_sb = cpool.tile([NG, GR * CROP], mybir.dt.float32)
            nc.gpsimd.dma_start(out=wrow_sb[:], in_=wraprows[:])
            selw_sb = cpool.tile([NG, BLK * C], mybir.dt.float32)
            nc.gpsimd.dma_start(out=selw_sb[:], in_=selwrap[:])
            mrow_sb = cpool.tile([1, FREE], mybir.dt.float32)
            nc.gpsimd.dma_start(out=mrow_sb[:], in_=maskrow[:])
            ones_sb = cpool.tile([1, BLK * C], mybir.dt.float32)
            nc.gpsimd.dma_start(out=ones_sb[:], in_=ones1[:])
            g_reg = nc.gpsimd.alloc_register("o_reg_pool")
            # SWDGE wrap reads for the last NWRAP samples, issued first: one
            # 128-partition DMA per sample (p = c*NG + g), each descriptor a
            # contiguous 2328-element run covering rows 11g..11g+10 via the
            # row-wrap trick. SWDGE is otherwise idle until the first store.
            wrap_tiles = []
            for wi in range(NWRAP):
                s = BPC - NWRAP + wi
                tw = wrappool.tile([BLK * C, WRUN], mybir.dt.float32, tag=f"wrap{wi}")
                nc.gpsimd.reg_load(g_reg, offs_sb[0:1, s : s + 1])
                ovw = nc.gpsimd.snap(g_reg, donate=True, min_val=0, max_val=MAXEOFF)
                base = images[s, :, 0:CROP, 0:CROP]
                src = bass.AP(
                    tensor=base.tensor,
                    offset=ovw,
                    ap=[[CHSZ, C], [GR * IN, NG], [1, WRUN]],
                    dep_tracking_offset=s * IMSZ,
                )
                nc.gpsimd.dma_start(out=tw[:], in_=src)
                wrap_tiles.append(tw)
            # wrap-sample mask [p = c*NG+g, r*88+k] = prof[11g+r]*prof[k],
            # replicated across channels by a selection matmul (done first so
            # it is ready as soon as the wrap reads land)
            wmask_sb = cpool.tile([BLK * C, GR * CROP], mybir.dt.float32)
            pcol = 512
            for ci in range((GR * CROP + pcol - 1) // pcol):
                lo = ci * pcol
                w = min(pcol, GR * CROP - lo)
                pt = ppool.tile([BLK * C, pcol], mybir.dt.float32, tag="pmask")
                nc.tensor.matmul(
                    out=pt[:, 0:w],
                    lhsT=selw_sb[:],
                    rhs=wrow_sb[:, lo : lo + w],
                    start=True,
                    stop=True,
                )
                nc.vector.tensor_copy(out=wmask_sb[:, lo : lo + w], in_=pt[:, 0:w])
            # replicate the [1, 7744] mask row to all 128 partitions on-chip
            mask_sb = cpool.tile([BLK * C, FREE], mybir.dt.float32)
            for ci in range((FREE + pcol - 1) // pcol):
                lo = ci * pcol
                w = min(pcol, FREE - lo)
                pt = ppool.tile([BLK * C, pcol], mybir.dt.float32, tag="pmask")
                nc.tensor.matmul(
                    out=pt[:, 0:w],
                    lhsT=ones_sb[0:1, :],
                    rhs=mrow_sb[0:1, lo : lo + w],
                    start=True,
                    stop=True,
                )
                nc.vector.tensor_copy(out=mask_sb[:, lo : lo + w], in_=pt[:, 0:w])
            # masked in-place compaction + store of the wrap samples:
            # crop (i=11g+r, k) sits at free offset r*224+k of partition
            # (c,g); write offset 88r+k never overtakes the read offset
            for wi in range(NWRAP):
                s = BPC - NWRAP + wi
                tw = wrap_tiles[wi]
                tw_ap = tw[:]
                tw_crop = bass.AP(
                    tensor=tw_ap.tensor,
                    offset=tw_ap.offset,
                    ap=[tw_ap.ap[0], [IN, GR], [1, CROP]],
                )
                nc.vector.tensor_tensor(
                    out=tw[:, 0 : GR * CROP], in0=tw_crop, in1=wmask_sb[:],
                    op=mybir.AluOpType.mult,
                )
                dstw = bass.AP(
                    tensor=out[:].tensor,
                    offset=s * C * FREE,
                    ap=[[FREE, C], [GR * CROP, NG], [1, GR * CROP]],
                )
                nc.gpsimd.dma_start(out=dstw, in_=tw[:, 0 : GR * CROP])

            regs = {
                "sync": nc.sync.alloc_register("o_reg_sp"),
                "scalar": nc.scalar.alloc_register("o_reg_act"),
            }
            engs = {"sync": nc.sync, "scalar": nc.scalar}
            # HWDGE chunks: samples 0-7 then 8-13 (14-15 went via SWDGE)
            chunks = [list(range(BLK)), list(range(BLK, BPC - NWRAP))]
            for blk, samples in enumerate(chunks):
                ns = len(samples)
                t = wpool.tile([ns * C, FREE], mybir.dt.float32, tag="blk")
                for j, s in enumerate(samples):
                    base = images[s, :, 0:CROP, 0:CROP]
                    split = len(samples) % 2 == 1 and j == len(samples) - 1
                    if split:
                        # odd sample count: split the last sample's read into
                        # row halves, one per ring, to keep the rings balanced
                        hrow = CROP // 2
                        for hi, rk in enumerate(("sync", "scalar")):
                            eng_, reg_ = engs[rk], regs[rk]
                            eng_.reg_load(reg_, offs_sb[0:1, s : s + 1])
                            ov = eng_.snap(
                                reg_, donate=True, min_val=0, max_val=MAXEOFF
                            )
                            srcap = bass.AP(
                                tensor=base.tensor,
                                offset=ov + hi * hrow * IN,
                                ap=[[CHSZ, C], [IN, hrow], [1, CROP]],
                                dep_tracking_offset=s * IMSZ,
                            )
                            eng_.dma_start(
                                out=t[
                                    j * C : (j + 1) * C,
                                    hi * hrow * CROP : (hi + 1) * hrow * CROP,
                                ],
                                in_=srcap,
                            )
                        continue
                    rk = "sync" if j % 2 == 0 else "scalar"
                    eng_, reg_ = engs[rk], regs[rk]
                    eng_.reg_load(reg_, offs_sb[0:1, s : s + 1])
                    ov = eng_.snap(reg_, donate=True, min_val=0, max_val=MAXEOFF)
                    srcap = bass.AP(
                        tensor=base.tensor,
                        offset=ov,
                        ap=[[CHSZ, C], [IN, CROP], [1, CROP]],
                        dep_tracking_offset=s * IMSZ,
                    )
                    eng_.dma_start(out=t[j * C : (j + 1) * C, :], in_=srcap)
                out_view = out[samples[0] : samples[0] + ns].rearrange(
                    "b c i k -> (b c) (i k)"
                )
                if blk == len(chunks) - 1:
                    # last chunk: halve the multiply along the free dim so the
                    # first half's store starts earlier; the final half's
                    # store is split across both idle HWDGE rings
                    fh = FREE // 2
                    for hi in range(2):
                        sl = slice(hi * fh, (hi + 1) * fh)
                        nc.vector.tensor_tensor(
                            out=t[:, sl],
                            in0=t[:, sl],
                            in1=mask_sb[0 : ns * C, sl],
                            op=mybir.AluOpType.mult,
                        )
                        if hi == 0:
                            nc.sync.dma_start(out=out_view[:, sl], in_=t[:, sl])
                        else:
                            fq = fh // 2
                            nc.sync.dma_start(
                                out=out_view[:, fh : fh + fq],
                                in_=t[:, fh : fh + fq],
                            )
                            nc.scalar.dma_start(
                                out=out_view[:, fh + fq : FREE],
                                in_=t[:, fh + fq : FREE],
                            )
                else:
                    nc.vector.tensor_tensor(
                        out=t[:], in0=t[:], in1=mask_sb[0 : ns * C, :],
                        op=mybir.AluOpType.mult,
                    )
                    nc.gpsimd.dma_start(out=out_view, in_=t[:])
    nc.finalize()
    return nc


def _get_nc():
    if "nc" not in _nc_cache:
        _nc_cache["nc"] = _build_nc()
    return _nc_cache["nc"]


def _host_offsets(locs):
    locs = np.asarray(locs, dtype=np.float32)
    t = np.clip(locs * np.float32(SCALE), np.float32(TL), np.float32(IN - TL))
    return np.floor(t - np.float32(TL)).astype(np.int32)  # [B, 2] (w, h)


def _host_mask():
    rr = np.arange(CROP, dtype=np.float32)
    sig = lambda z: (1.0 / (1.0 + np.exp(-10.0 * z, dtype=np.float32))).astype(
        np.float32
    )
    prof = sig(rr) - sig(rr - np.float32(CROP))
    mask = np.outer(prof, prof).astype(np.float32)  # [88, 88]
    maskrow = np.ascontiguousarray(mask.reshape(1, -1))
    # wraprows[g, r*88+k] = mask[11g+r, k]
    wraprows = np.ascontiguousarray(mask.reshape(NG, GR * CROP))
    # selwrap[g, p] = 1 where p % NG == g  (partition p = c*NG + g)
    selwrap = np.zeros((NG, BLK * C), dtype=np.float32)
    for g in range(NG):
        selwrap[g, g::NG] = 1.0
    return maskrow, wraprows, selwrap


def make_in_maps(images, locs):
    images = np.asarray(images, dtype=np.float32)
    off = _host_offsets(locs)  # [B, 2] (w, h)
    s_idx = np.arange(BPC, dtype=np.int64)
    maskrow, wraprows, selwrap = _host_mask()
    ones1 = np.ones((1, BLK * C), dtype=np.float32)
    in_maps = []
    for i in range(NCORES):
        sl = slice(i * BPC, (i + 1) * BPC)
        osh = off[sl].astype(np.int64)
        eoff = (s_idx * IMSZ + osh[:, 0] * IN + osh[:, 1]).astype(np.int32)
        in_maps.append(
            {
                "images": np.ascontiguousarray(images[sl]),
                "offs": np.ascontiguousarray(eoff.reshape(1, -1)),
                "maskrow": maskrow,
                "ones1": ones1,
                "wraprows": wraprows,
                "selwrap": selwrap,
            }
        )
    return in_maps


def run(images, locs, trace=False, **kwargs):
    nc = _get_nc()
    in_maps = make_in_maps(images, locs)
    res = run_bass_kernel_spmd(
        nc, in_maps, core_ids=list(range(NCORES)), trace=trace, **kwargs
    )
    outs = [np.asarray(res.results[i]["out"]) for i in range(NCORES)]
    full = np.concatenate(outs, axis=0).astype(np.float32)
    return full, res


def kernel(images, locs):
    full, _ = run(images, locs, trace=False)
    return full


# revision 73
# speedup vs baseline: 1.0977x; 1.0977x over previous
"""AttentionCropLayer Trainium2 kernel.

Per sample b: offsets (w,h) = floor(clip(locs[b]*224, 44, 180) - 44); output
out[b] = images[b, :, w:w+88, h:h+88] * mask, with mask the fixed 88x88
sigmoid-profile outer product.

Strategy (pure data parallel, 8 cores x 16 samples):
  - host precomputes per-sample flat element offsets (exact same f32 ops as
    the reference) plus tiny constant tables for the mask
  - device, per chunk of 8 samples (=128 partitions, partition = (sample,
    channel)): per sample one dynamic-offset HWDGE read DMA (samples
    alternate the two HWDGE rings so every SDMA engine interleaves two
    descriptor streams), each descriptor an exact 352B crop row; per chunk
    one in-place DVE mask-multiply (mask replicated on-chip via an
    outer-product matmul) and one contiguous 3.96MB store
"""

import sys

if "/opt/trn_rl_repo" not in sys.path:
    sys.path.insert(0, "/opt/trn_rl_repo")

import numpy as np

import concourse.bass as bass
import concourse.bacc as bacc
import concourse.mybir as mybir
from concourse import tile
from concourse.bass_utils import run_bass_kernel_spmd

TL = 44
CROP = 2 * TL          # 88
SCALE = 224.0
B, C, IN = 128, 16, 224
NCORES = 8
BPC = B // NCORES      # 16 samples per core
BLK = 8                # samples per chunk -> BLK*C = 128 partitions
NBLK = BPC // BLK      # 2 chunks per core
FREE = CROP * CROP     # 7744
NG = 8                 # row groups for the SWDGE wrap samples (8*16 = 128)
GR = CROP // NG        # 11 rows per group
WRUN = (GR - 1) * IN + CROP  # 2328: contiguous wrap run per (c, g)
NWRAP = 2              # samples read via SWDGE wrap layout
MAXOFF = IN - CROP     # 136
IMSZ = C * IN * IN
CHSZ = IN * IN
MAXEOFF = (BPC - 1) * IMSZ + MAXOFF * IN + MAXOFF

_nc_cache = {}


def _build_nc():
    nc = bacc.Bacc(None)
    images = nc.declare_dram_parameter(
        "images", [BPC, C, IN, IN], mybir.dt.float32, isOutput=False
    )
    offs = nc.declare_dram_parameter(
        "offs", [1, BPC], mybir.dt.int32, isOutput=False
    )
    maskrow = nc.declare_dram_parameter(
        "maskrow", [1, FREE], mybir.dt.float32, isOutput=False
    )
    ones1 = nc.declare_dram_parameter(
        "ones1", [1, BLK * C], mybir.dt.float32, isOutput=False
    )
    # tables for the two SWDGE wrap-read samples: partition p = c*NG + g
    wraprows = nc.declare_dram_parameter(
        "wraprows", [NG, GR * CROP], mybir.dt.float32, isOutput=False
    )
    selwrap = nc.declare_dram_parameter(
        "selwrap", [NG, BLK * C], mybir.dt.float32, isOutput=False
    )
    out = nc.declare_dram_parameter(
        "out", [BPC, C, CROP, CROP], mybir.dt.float32, isOutput=True
    )

    with tile.TileContext(nc) as tc:
        with (
            tc.tile_pool(name="const", bufs=1) as cpool,
            tc.tile_pool(name="work", bufs=2) as wpool,
            tc.tile_pool(name="wrapp", bufs=1) as wrappool,
            tc.tile_pool(name="psum", bufs=2, space="PSUM") as ppool,
        ):
            offs_sb = cpool.tile([1, BPC], mybir.dt.int32)
            nc.sync.dma_start(out=offs_sb[:], in_=offs[:])
            wrow_sb = cpool.tile([NG, GR * CROP], mybir.dt.float32)
            nc.gpsimd.dma_start(out=wrow_sb[:], in_=wraprows[:])
            selw_sb = cpool.tile([NG, BLK * C], mybir.dt.float32)
            nc.gpsimd.dma_start(out=selw_sb[:], in_=selwrap[:])
            mrow_sb = cpool.tile([1, FREE], mybir.dt.float32)
            nc.gpsimd.dma_start(out=mrow_sb[:], in_=maskrow[:])
            ones_sb = cpool.tile([1, BLK * C], mybir.dt.float32)
            nc.gpsimd.dma_start(out=ones_sb[:], in_=ones1[:])
            g_reg = nc.gpsimd.alloc_register("o_reg_pool")
            # SWDGE wrap reads for the last NWRAP samples, issued first: one
            # 128-partition DMA per sample (p = c*NG + g), each descriptor a
            # contiguous 2328-element run covering rows 11g..11g+10 via the
            # row-wrap trick. SWDGE is otherwise idle until the first store.
            wrap_tiles = []
            for wi in range(NWRAP):
                s = BPC - NWRAP + wi
                tw = wrappool.tile([BLK * C, WRUN], mybir.dt.float32, tag=f"wrap{wi}")
                nc.gpsimd.reg_load(g_reg, offs_sb[0:1, s : s + 1])
                ovw = nc.gpsimd.snap(g_reg, donate=True, min_val=0, max_val=MAXEOFF)
                base = images[s, :, 0:CROP, 0:CROP]
                src = bass.AP(
                    tensor=base.tensor,
                    offset=ovw,
                    ap=[[CHSZ, C], [GR * IN, NG], [1, WRUN]],
                    dep_tracking_offset=s * IMSZ,
                )
                nc.gpsimd.dma_start(out=tw[:], in_=src)
                wrap_tiles.append(tw)
            # wrap-sample mask [p = c*NG+g, r*88+k] = prof[11g+r]*prof[k],
            # replicated across channels by a selection matmul (done first so
            # it is ready as soon as the wrap reads land)
            wmask_sb = cpool.tile([BLK * C, GR * CROP], mybir.dt.float32)
            pcol = 512
            for ci in range((GR * CROP + pcol - 1) // pcol):
                lo = ci * pcol
                w = min(pcol, GR * CROP - lo)
                pt = ppool.tile([BLK * C, pcol], mybir.dt.float32, tag="pmask")
                nc.tensor.matmul(
                    out=pt[:, 0:w],
                    lhsT=selw_sb[:],
                    rhs=wrow_sb[:, lo : lo + w],
                    start=True,
                    stop=True,
                )
                nc.vector.tensor_copy(out=wmask_sb[:, lo : lo + w], in_=pt[:, 0:w])
            # replicate the [1, 7744] mask row to all 128 partitions on-chip
            mask_sb = cpool.tile([BLK * C, FREE], mybir.dt.float32)
            for ci in range((FREE + pcol - 1) // pcol):
                lo = ci * pcol
                w = min(pcol, FREE - lo)
                pt = ppool.tile([BLK * C, pcol], mybir.dt.float32, tag="pmask")
                nc.tensor.matmul(
                    out=pt[:, 0:w],
                    lhsT=ones_sb[0:1, :],
                    rhs=mrow_sb[0:1, lo : lo + w],
                    start=True,
                    stop=True,
                )
                nc.vector.tensor_copy(out=mask_sb[:, lo : lo + w], in_=pt[:, 0:w])
            # masked in-place compaction + store of the wrap samples:
            # crop (i=11g+r, k) sits at free offset r*224+k of partition
            # (c,g); write offset 88r+k never overtakes the read offset
            for wi in range(NWRAP):
                s = BPC - NWRAP + wi
                tw = wrap_tiles[wi]
                tw_ap = tw[:]
                tw_crop = bass.AP(
                    tensor=tw_ap.tensor,
                    offset=tw_ap.offset,
                    ap=[tw_ap.ap[0], [IN, GR], [1, CROP]],
                )
                nc.vector.tensor_tensor(
                    out=tw[:, 0 : GR * CROP], in0=tw_crop, in1=wmask_sb[:],
                    op=mybir.AluOpType.mult,
                )
                dstw = bass.AP(
                    tensor=out[:].tensor,
                    offset=s * C * FREE,
                    ap=[[FREE, C], [GR * CROP, NG], [1, GR * CROP]],
                )
                nc.gpsimd.dma_start(out=dstw, in_=tw[:, 0 : GR * CROP])

            regs = {
                "sync": nc.sync.alloc_register("o_reg_sp"),
                "scalar": nc.scalar.alloc_register("o_reg_act"),
            }
            engs = {"sync": nc.sync, "scalar": nc.scalar}
            # HWDGE chunks: samples 0-7 then 8-13 (14-15 went via SWDGE)
            chunks = [list(range(BLK)), list(range(BLK, BPC - NWRAP))]
            for blk, samples in enumerate(chunks):
                ns = len(samples)
                t = wpool.tile([ns * C, FREE], mybir.dt.float32, tag="blk")
                for j, s in enumerate(samples):
                    base = images[s, :, 0:CROP, 0:CROP]
                    split = len(samples) % 2 == 1 and j == len(samples) - 1
                    if split:
                        # odd sample count: split the last sample's read into
                        # row halves, one per ring, to keep the rings balanced
                        hrow = CROP // 2
                        for hi, rk in enumerate(("sync", "scalar")):
                            eng_, reg_ = engs[rk], regs[rk]
                            eng_.reg_load(reg_, offs_sb[0:1, s : s + 1])
                            ov = eng_.snap(
                                reg_, donate=True, min_val=0, max_val=MAXEOFF
                            )
                            srcap = bass.AP(
                                tensor=base.tensor,
                                offset=ov + hi * hrow * IN,
                                ap=[[CHSZ, C], [IN, hrow], [1, CROP]],
                                dep_tracking_offset=s * IMSZ,
                            )
                            eng_.dma_start(
                                out=t[
                                    j * C : (j + 1) * C,
                                    hi * hrow * CROP : (hi + 1) * hrow * CROP,
                                ],
                                in_=srcap,
                            )
                        continue
                    rk = "sync" if j % 2 == 0 else "scalar"
                    eng_, reg_ = engs[rk], regs[rk]
                    eng_.reg_load(reg_, offs_sb[0:1, s : s + 1])
                    ov = eng_.snap(reg_, donate=True, min_val=0, max_val=MAXEOFF)
                    srcap = bass.AP(
                        tensor=base.tensor,
                        offset=ov,
                        ap=[[CHSZ, C], [IN, CROP], [1, CROP]],
                        dep_tracking_offset=s * IMSZ,
                    )
                    eng_.dma_start(out=t[j * C : (j + 1) * C, :], in_=srcap)
                out_view = out[samples[0] : samples[0] + ns].rearrange(
                    "b c i k -> (b c) (i k)"
                )
                if blk == len(chunks) - 1:
                    # last chunk: halve the multiply along the free dim so the
                    # first half's store starts earlier, and put both stores
                    # on the now-idle HWDGE rings
                    fh = FREE // 2
                    for hi, weng in enumerate((nc.sync, nc.scalar)):
                        sl = slice(hi * fh, (hi + 1) * fh)
                        nc.vector.tensor_tensor(
                            out=t[:, sl],
                            in0=t[:, sl],
                            in1=mask_sb[0 : ns * C, sl],
                            op=mybir.AluOpType.mult,
                        )
                        weng.dma_start(out=out_view[:, sl], in_=t[:, sl])
                else:
                    nc.vector.tensor_tensor(
                        out=t[:], in0=t[:], in1=mask_sb[0 : ns * C, :],
                        op=mybir.AluOpType.mult,
                    )
                    nc.gpsimd.dma_start(out=out_view, in_=t[:])
    nc.finalize()
    return nc


def _get_nc():
    if "nc" not in _nc_cache:
        _nc_cache["nc"] = _build_nc()
    return _nc_cache["nc"]


def _host_offsets(locs):
    locs = np.asarray(locs, dtype=np.float32)
    t = np.clip(locs * np.float32(SCALE), np.float32(TL), np.float32(IN - TL))
    return np.floor(t - np.float32(TL)).astype(np.int32)  # [B, 2] (w, h)


def _host_mask():
    rr = np.arange(CROP, dtype=np.float32)
    sig = lambda z: (1.0 / (1.0 + np.exp(-10.0 * z, dtype=np.float32))).astype(
        np.float32
    )
    prof = sig(rr) - sig(rr - np.float32(CROP))
    mask = np.outer(prof, prof).astype(np.float32)  # [88, 88]
    maskrow = np.ascontiguousarray(mask.reshape(1, -1))
    # wraprows[g, r*88+k] = mask[11g+r, k]
    wraprows = np.ascontiguousarray(mask.reshape(NG, GR * CROP))
    # selwrap[g, p] = 1 where p % NG == g  (partition p = c*NG + g)
    selwrap = np.zeros((NG, BLK * C), dtype=np.float32)
    for g in range(NG):
        selwrap[g, g::NG] = 1.0
    return maskrow, wraprows, selwrap


def make_in_maps(images, locs):
    images = np.asarray(images, dtype=np.float32)
    off = _host_offsets(locs)  # [B, 2] (w, h)
    s_idx = np.arange(BPC, dtype=np.int64)
    maskrow, wraprows, selwrap = _host_mask()
    ones1 = np.ones((1, BLK * C), dtype=np.float32)
    in_maps = []
    for i in range(NCORES):
        sl = slice(i * BPC, (i + 1) * BPC)
        osh = off[sl].astype(np.int64)
        eoff = (s_idx * IMSZ + osh[:, 0] * IN + osh[:, 1]).astype(np.int32)
        in_maps.append(
            {
                "images": np.ascontiguousarray(images[sl]),
                "offs": np.ascontiguousarray(eoff.reshape(1, -1)),
                "maskrow": maskrow,
                "ones1": ones1,
                "wraprows": wraprows,
                "selwrap": selwrap,
            }
        )
    return in_maps


def run(images, locs, trace=False, **kwargs):
    nc = _get_nc()
    in_maps = make_in_maps(images, locs)
    res = run_bass_kernel_spmd(
        nc, in_maps, core_ids=list(range(NCORES)), trace=trace, **kwargs
    )
    outs = [np.asarray(res.results[i]["out"]) for i in range(NCORES)]
    full = np.concatenate(outs, axis=0).astype(np.float32)
    return full, res


def kernel(images, locs):
    full, _ = run(images, locs, trace=False)
    return full
